# revision 1
# baseline (speedup 1.0000x reference)
"""Trainium2 Bass kernel: ConvTranspose3d(32->64,k3,s2,p1) + 0.5x + MaxPool3d(2) +
global-avg-pool + clamp(0,1), data-parallel over batch on 8 NeuronCores.

Math: a stride-2 transposed conv splits into 8 parity classes (even/odd output
index per spatial axis). Each 2x2x2 maxpool window holds exactly one output of
each class, so maxpool == elementwise max over the 8 class sub-convolutions.
Floor-mode pooling drops conv row 30 (d) / 62 (h,w), so every needed tap is
interior: no zero padding anywhere.

Per axis: even output 2j uses kernel tap k=1 with x[j]; odd output 2j+1 uses
k=2 with x[j] and k=0 with x[j+1]. So every tap's input is x shifted by
(dd,dh,dw) in {0,1}^3. We keep 4 shifted copies of x ((dh,dw) combos) in the
4 32-partition blocks of one SBUF tile; the dd shift is a free-dim offset of
D_stride. A matmul pass contracts K = 32*c_in x (up to 4 taps) at once, and
packs 2 classes into M=128 (64 out-channels each). 6 passes cover all
27 taps x 8 classes; classes are paired into 4 PSUM banks so the two classes
of a bank are the two partition halves.
"""

import numpy as np

import concourse.bass as bass
import concourse.bacc as bacc
import concourse.mybir as mybir
from concourse.tile import TileContext
from concourse.bass_utils import run_bass_kernel_spmd
from concourse.alu_op_type import AluOpType

# Problem constants (hardcoded per contract)
N_BATCH = 8
IN_C, OUT_C = 32, 64
D, H, W = 16, 32, 32
JD, JH, JW = 15, 31, 31          # pooled output grid
NPOS = JD * JH * JW              # 14415
SCALE = 0.5
FREE = D * H * W                 # 16384 flat free size per c_in
DSTR, HSTR = H * W, W            # flat strides

# SBUF x-tile block shifts (dh, dw) for partition blocks 0..3.
# Order chosen so every class's tap set is a contiguous block range.
BLOCKS = [(0, 1), (0, 0), (1, 0), (1, 1)]
BLOCK_OFF = [dh * HSTR + dw for (dh, dw) in BLOCKS]   # 32, 0, ... -> [33?]

# Chunking of the pooled grid: (jd, h0, hcnt): hcnt h-rows x 31 w positions
# per chunk (bf16 matmuls allow strided 3D moving APs, so only valid pooled
# positions are ever computed). CN[i] = chunk free size; CO[i] = col offset of
# chunk i within its pair tile.
CHUNKS = [(jd, h0, hcnt) for jd in range(JD) for (h0, hcnt) in ((0, 16), (16, 15))]
CN = [hcnt * JW for (_, _, hcnt) in CHUNKS]
PAIRW = CN[0] + CN[1]   # 961 cols per pair tile
NPAIR = len(CHUNKS) // 2

# Matmul passes: (pass_idx, bank, dd, start, stop).
# Bank pairing (partition half 0 / half 1):
#   bank0: (1,1,1) | (1,0,0)     bank1: (1,0,1) | (1,1,0)
#   bank2: (0,0,0) | (0,0,1)     bank3: (0,1,0) | (0,1,1)
PASSES = [
    (1, 0, 0, 0, True, False),   # bank1 -> bpC slot0
    (5, 0, 0, 1, False, True),
    (3, 0, 1, 0, True, True),    # bank3 -> bpC slot1
    (0, 1, 0, 0, True, False),   # bank0 -> bpM slot0
    (4, 1, 0, 1, False, True),
    (2, 1, 1, 0, True, True),    # bank2 -> bpM slot1
]
PASSES_HEAD = [PASSES[0], PASSES[3], PASSES[2], PASSES[5], PASSES[1], PASSES[4]]
assert [p[3] for p in PASSES_HEAD] == [0, 0, 0, 0, 1, 1]  # dd-major
NPASS = 6


def build_wstack(w: np.ndarray) -> np.ndarray:
    """Stack torch-layout ConvTranspose3d weights (in,out,kd,kh,kw) into the
    6 lhsT matrices, as one [128, 6*128] array: rows = 32*block + c_in,
    cols = 128*pass + 64*half + c_out. Unused rows stay zero."""
    wstk = np.zeros((128, NPASS * 128), np.float32)

    def fill(p, half, ph, pw, kd):
        for bidx, (dh, dw) in enumerate(BLOCKS):
            if dh > ph or dw > pw:
                continue  # block not in this class's tap set
            kh = 1 if ph == 0 else 2 - 2 * dh
            kw = 1 if pw == 0 else 2 - 2 * dw
            col = p * 128 + 64 * half
            wstk[32 * bidx: 32 * bidx + 32, col: col + OUT_C] = w[:, :, kd, kh, kw]

    # dd=0 passes: pd=1 classes use kd=2, pd=0 classes use kd=1
    fill(0, 0, 1, 1, 2); fill(0, 1, 0, 0, 2)
    fill(1, 0, 0, 1, 2); fill(1, 1, 1, 0, 2)
    fill(2, 0, 0, 0, 1); fill(2, 1, 0, 1, 1)
    fill(3, 0, 1, 0, 1); fill(3, 1, 1, 1, 1)
    # dd=1 passes: pd=1 classes, kd=0
    fill(4, 0, 1, 1, 0); fill(4, 1, 0, 0, 0)
    fill(5, 0, 0, 1, 0); fill(5, 1, 1, 0, 0)
    return wstk


def build_nc() -> bass.Bass:
    # Bacc (not raw Bass): its compile pipeline splits multi-sem waits and
    # moves matmul waits to ldweights to satisfy the 1-wait-per-instruction
    # hardware constraint.
    nc = bacc.Bacc()
    f32 = mybir.dt.float32
    bf16 = mybir.dt.bfloat16

    x_d = nc.declare_dram_parameter("x", [IN_C, FREE], bf16, isOutput=False)
    w_d = nc.declare_dram_parameter("wstk", [128, NPASS * 128], bf16, isOutput=False)
    b_d = nc.declare_dram_parameter("bvec", [OUT_C, 1], f32, isOutput=False)
    o_d = nc.declare_dram_parameter("out", [OUT_C, 1], f32, isOutput=True)

    with TileContext(nc) as tc:
        with (
            tc.tile_pool(name="xp", bufs=1) as xp,
            tc.tile_pool(name="wp", bufs=1) as wp,
            tc.tile_pool(name="ps", bufs=4, space="PSUM") as ps,
            tc.tile_pool(name="mp", bufs=8) as mp,
            tc.tile_pool(name="ap", bufs=1) as ap,
        ):
            wt = wp.tile([128, NPASS * 128], bf16, tag="wt")
            nc.scalar.dma_start(out=wt[:, 0:256], in_=w_d[:, 0:256])

            xbuf = xp.tile([128, FREE], bf16, tag="x")
            # 4 shifted copies of x, 2 big DMAs per block (per-DMA dispatch on
            # the SP sequencer is expensive; keep the count low but still let
            # the first half arrive before the second finishes).
            # Small priority slab (d-rows 0-2) per block first, so the first
            # chunk pair's matmuls start ~10us earlier than with halves only.
            SLAB = 2 * DSTR
            HALF = FREE // 2
            for bidx, off in enumerate(BLOCK_OFF):
                eng = nc.sync if bidx % 2 == 0 else nc.scalar
                eng.dma_start(
                    out=xbuf[32 * bidx: 32 * bidx + 32, 0:SLAB],
                    in_=x_d[:, off: off + SLAB],
                )
            nc.scalar.dma_start(out=wt[:, 256:], in_=w_d[:, 256:])
            for bidx, off in enumerate(BLOCK_OFF):
                nc.sync.dma_start(
                    out=xbuf[32 * bidx: 32 * bidx + 32, SLAB:HALF],
                    in_=x_d[:, SLAB + off: off + HALF],
                )
            for bidx, off in enumerate(BLOCK_OFF):
                ln = FREE - HALF - 33
                nc.sync.dma_start(
                    out=xbuf[32 * bidx: 32 * bidx + 32, HALF: HALF + ln],
                    in_=x_d[:, HALF + off: HALF + off + ln],
                )

            bv = wp.tile([OUT_C, 1], f32, tag="bv")
            nc.sync.dma_start(out=bv[:, :], in_=b_d[:, :])

            xv = xbuf[:, :].rearrange("p (d h w) -> p d h w", d=D, h=H, w=W)
            acc = ap.tile([OUT_C, len(CHUNKS) // 2 + 1], f32, tag="acc")

            def consume_pair(cbase, banksets):
                """Reduce 2 chunks x 2 bank-pair supertiles (8 classes) into a
                [128, 961] pair tile mm. Per chunk: one 2-bank-wide 3D-AP ACT
                copy of the bpC tile; mixed chunks then run DVE maxes against
                the bpM PSUM slices, heavy chunks ACT-copy bpM too and max
                all-SBUF in 2x mode. Pair-level halves fold via HWDGE DMA,
                deferred one pair."""
                mm = mp.tile([128, PAIRW], bf16, name="mm", tag="mm")
                half_dmas = []
                for k, (bpC, bpM) in enumerate(banksets):
                    n = CN[cbase + k]
                    o = 0 if k == 0 else CN[cbase]
                    ci = cbase + k
                    cc = mp.tile([128, 2 * 496], bf16, name="cc", tag="cc")
                    ccv = cc[:, :].rearrange("p (b n) -> p b n", b=2, n=496)
                    bCv = bpC[:, :].rearrange("p (b n) -> p b n", b=2, n=512)
                    nc.scalar.copy(ccv[:, :, 0:n], bCv[:, :, 0:n])
                    m01 = mp.tile([128, 496], bf16, name="m01", tag="m01")
                    m23 = mp.tile([128, 496], bf16, name="m23", tag="m23")
                    if ci == 7:
                        # p1 chunk: copy only bpM slot0; one mixed max — a
                        # half-step between mixed and heavy for fine balance
                        cm = mp.tile([128, 2 * 496], bf16, name="cm", tag="cm")
                        cmv = cm[:, :].rearrange("p (b n) -> p b n", b=2, n=496)
                        nc.scalar.copy(cmv[:, 0, 0:n], bpM[:, 0:n])
                        nc.vector.tensor_max(m01[:, :n], ccv[:, 0, 0:n], cmv[:, 0, 0:n])
                        nc.vector.tensor_max(m23[:, :n], ccv[:, 1, 0:n], bpM[:, 512:512 + n])
                    elif ci % 2 == 1 or ci in (14, 22, 28):
                        # ACT-heavy chunk: 2-bank copy of bpM too; DVE maxes
                        # run all-SBUF in 2x mode.
                        cm = mp.tile([128, 2 * 496], bf16, name="cm", tag="cm")
                        cmv = cm[:, :].rearrange("p (b n) -> p b n", b=2, n=496)
                        bMv = bpM[:, :].rearrange("p (b n) -> p b n", b=2, n=512)
                        nc.scalar.copy(cmv[:, :, 0:n], bMv[:, :, 0:n])
                        nc.vector.tensor_max(m01[:, :n], ccv[:, 0, 0:n], cmv[:, 0, 0:n])
                        nc.vector.tensor_max(m23[:, :n], ccv[:, 1, 0:n], cmv[:, 1, 0:n])
                    else:
                        nc.vector.tensor_max(m01[:, :n], ccv[:, 0, 0:n], bpM[:, 0:n])
                        nc.vector.tensor_max(m23[:, :n], ccv[:, 1, 0:n], bpM[:, 512:512 + n])
                    nc.vector.tensor_max(
                        mm[:, o: o + n], m01[:, :n], m23[:, :n]
                    )
                    if cbase + 2 >= len(CHUNKS):
                        mhalf = mp.tile([OUT_C, 496], bf16,
                                        name=f"mhalf{k}", tag=f"mhalf{k}")
                        nc.sync.dma_start(
                            out=mhalf[:, 0:n], in_=mm[OUT_C:128, o: o + n]
                        )
                        half_dmas.append((mhalf, o, n))
                # pair-level fold, deferred one pair (the shift DMA's
                # ~2.5us latency overlaps the next pair's stage 1 instead of
                # stalling the DVE FIFO). The last pair folds per half, each
                # shift DMA dispatched the moment its half of mm is complete.
                if cbase + 2 >= len(CHUNKS):
                    last_half_dmas.append(half_dmas)
                    return mm, None
                mhi = mp.tile([OUT_C, PAIRW], bf16, name="mhi", tag="mhi")
                nc.sync.dma_start(out=mhi[:, :], in_=mm[OUT_C:128, :])
                return mm, mhi

            def fold_reduce(pidx, mm, mhi):
                mh = mp.tile([OUT_C, PAIRW], bf16, name="mh", tag="mh")
                mg = mp.tile([OUT_C, PAIRW], bf16, name="mg", tag="mg")
                nc.vector.tensor_max(mh[:, :], mm[0:OUT_C, :], mhi[:, :])
                if NPAIR - 4 <= pidx < NPAIR - 1:
                    # tail pairs: ACT is idle there while DVE is saturated
                    nc.scalar.activation(
                        mg[:, :], mh[:, :],
                        mybir.ActivationFunctionType.Copy,
                        accum_out=acc[:, pidx: pidx + 1],
                    )
                else:
                    nc.vector.tensor_scalar(
                        mg[:, :], mh[:, :], 1.0, None,
                        op0=AluOpType.mult, op1=AluOpType.add,
                        accum_out=acc[:, pidx: pidx + 1],
                    )

            # Process chunks in pairs so the two matmuls sharing an lhsT are
            # back-to-back (weight-load reuse) and PSUM double-buffers.
            pending = []
            last_half_dmas = []
            for cbase in range(0, len(CHUNKS), 2):
                pair = CHUNKS[cbase: cbase + 2]
                banksets = []
                for k in range(len(pair)):
                    banksets.append((
                        ps.tile([128, 1024], f32, name=f"bpC_{k}", tag="bp"),
                        ps.tile([128, 1024], f32, name=f"bpM_{k}", tag="bp"),
                    ))
                passes = PASSES
                for (p, tile_i, slot, dd, start, stop) in passes:
                    for k, (jd, h0, hcnt) in enumerate(pair):
                        n = hcnt * JW
                        rhs = xv[:, jd + dd, h0: h0 + hcnt, 0:JW]
                        nc.tensor.matmul(
                            banksets[k][tile_i][:, slot * 512: slot * 512 + n],
                            wt[:, p * 128: (p + 1) * 128],
                            rhs,
                            start=start,
                            stop=stop,
                        )
                mm_mhi = consume_pair(cbase, banksets)
                if len(pending) >= 2:
                    fold_reduce(*pending.pop(0))
                pending.append((cbase // 2, *mm_mhi))
            while len(pending) > 1:
                fold_reduce(*pending.pop(0))
            pidx, mm, halves = pending[0][0], pending[0][1], last_half_dmas[0]
            for hi, (mhalf, o, n) in enumerate(halves):
                mh = mp.tile([OUT_C, 496], bf16, name=f"lmh{hi}", tag=f"lmh{hi}")
                mg = mp.tile([OUT_C, 496], bf16, name=f"lmg{hi}", tag=f"lmg{hi}")
                nc.vector.tensor_max(
                    mh[:, 0:n], mm[0:OUT_C, o: o + n], mhalf[:, 0:n]
                )
                nc.vector.tensor_scalar(
                    mg[:, 0:n], mh[:, 0:n], 1.0, None,
                    op0=AluOpType.mult, op1=AluOpType.add,
                    accum_out=acc[:, pidx + hi: pidx + hi + 1],
                )

            tot = ap.tile([OUT_C, 1], f32, tag="tot")
            nc.vector.reduce_sum(tot[:, :], acc[:, :], axis=mybir.AxisListType.X)
            res = ap.tile([OUT_C, 1], f32, tag="res")
            # res = clamp(tot * (SCALE/NPOS) + 0.5*b, 0, 1); bvec is pre-scaled.
            nc.vector.scalar_tensor_tensor(
                res[:, :], tot[:, :], SCALE / NPOS, bv[:, :],
                op0=AluOpType.mult, op1=AluOpType.add,
            )
            out_t = ap.tile([OUT_C, 1], f32, tag="outt")
            nc.vector.tensor_scalar(
                out_t[:, :], res[:, :], 0.0, 1.0,
                op0=AluOpType.max, op1=AluOpType.min,
            )
            nc.sync.dma_start(out=o_d[:, :], in_=out_t[:, :])

    return nc


_NC_CACHE = None


def _get_nc():
    global _NC_CACHE
    if _NC_CACHE is None:
        _NC_CACHE = build_nc()
        # Bacc.finalize runs the wait-splitting/register-allocation pipeline;
        # the pjrt exec path requires a finalized module.
        _NC_CACHE.finalize()
    return _NC_CACHE


def run(x, w, b, **spmd_kwargs):
    """Run on 8 cores; returns (output (8,64,1,1,1), BassKernelResults)."""
    import ml_dtypes
    bf = np.dtype(ml_dtypes.bfloat16)
    x = np.ascontiguousarray(x, np.float32)
    wstk = build_wstack(np.asarray(w, np.float32)).astype(bf)
    bvec = (SCALE * np.asarray(b, np.float32)).reshape(OUT_C, 1).copy()
    nc = _get_nc()
    in_maps = [
        {"x": x[i].reshape(IN_C, FREE).astype(bf), "wstk": wstk, "bvec": bvec}
        for i in range(N_BATCH)
    ]
    r = run_bass_kernel_spmd(nc, in_maps, list(range(N_BATCH)), **spmd_kwargs)
    out = np.stack(
        [np.asarray(r.results[i]["out"], np.float32).reshape(OUT_C) for i in range(N_BATCH)]
    )
    return out.reshape(N_BATCH, OUT_C, 1, 1, 1), r


def kernel(x, w, b):
    out, _ = run(x, w, b)
    return out

